# revision 6
# baseline (speedup 1.0000x reference)
"""Trainium2 Bass kernel for nn_Attention_55499567399068.

Episode-attention block: per (batch, nway) pair of [64, 512] blocks:
  q/k/v linear projections -> scaled dot-product attention over nshot ->
  reduce_att MLP producing per-row weights -> weighted sum of context rows.

Sharding: pure data parallel over batch across 8 NeuronCores (32 episodes each).
Per core: 256 independent (b, n) pairs.

Math restructuring (exact, up to fp precision):
  - 1/sqrt(d) folded into Wq, bq on host.
  - Attention kept UNNORMALIZED through the pipeline (E = exp(S), Z = rowsum(E)).
    Softmax normalization, reduce-MLP bias handling, and the final weighted sum
    all fold into per-pair 64-vectors:
       hid~ = Wr1 @ C~^T + br1 (x) Z        (rank-1 via a BR1 matmul, C~ = E @ v)
       leaky(hid~) = Z * leaky(hid)         (positive homogeneity)
       w^ = (w~ * r + br2) * r, r = 1/Z     (all normalization lands here)
       out^T[h] = sum_k v[k,h] * g[k], g = E^T @ w^
  - bq, bk added via per-partition activation bias on the PSUM->SBUF copy
    (q^T/k^T layouts have h on partitions); bv added via a broadcast-tile
    tensor_tensor add on the v copy (v natural layout has h on free).

Layout strategy: inputs are loaded naturally ([row, d]), cast to bf16, and
transposed to [d, row] with the DMA xbar (2-byte transpose engine) so the
d-contraction projections run at full PE rate.
"""

import sys

sys.path.insert(0, "/opt/trn_rl_repo")

import ml_dtypes
import numpy as np

import concourse.bass as bass
import concourse.tile as tile
from concourse import bacc, mybir
from concourse.bass_utils import run_bass_kernel_spmd

F32 = mybir.dt.float32
BF16 = mybir.dt.bfloat16
BF16_NP = ml_dtypes.bfloat16

BS, NWAY, NSHOT, D = 256, 8, 64, 512
NCORES = 8
BS_SH = BS // NCORES  # 32 episodes per core
NPAIR = BS_SH * NWAY  # 256 pairs per core
SUPER = 8  # pairs per superblock
NSB = NPAIR // SUPER  # 32 superblocks
ROWS_SB = SUPER * NSHOT  # 512 rows per superblock
LEAK = 0.01
AT = mybir.ActivationFunctionType
ALU = mybir.AluOpType


def build_nc(repeat=1, cast_dma=True, n_sb=NSB):
    nc = bacc.Bacc("TRN2", target_bir_lowering=False)

    # --- DRAM parameters (per-core shard shapes) ---
    xq = nc.dram_tensor("xq", [NPAIR * NSHOT, D], F32, kind="ExternalInput")
    xk = nc.dram_tensor("xk", [NPAIR * NSHOT, D], F32, kind="ExternalInput")
    xv = nc.dram_tensor("xv", [NPAIR * NSHOT, D], F32, kind="ExternalInput")
    wqT_d = nc.dram_tensor("wqT", [D, D], BF16, kind="ExternalInput")  # [d, h]
    wkT_d = nc.dram_tensor("wkT", [D, D], BF16, kind="ExternalInput")
    wvT_d = nc.dram_tensor("wvT", [D, D], BF16, kind="ExternalInput")
    wr1T_d = nc.dram_tensor("wr1T", [D, 64], BF16, kind="ExternalInput")  # [h, m]
    wr2T_d = nc.dram_tensor("wr2T", [64, 1], BF16, kind="ExternalInput")  # [m, 1]
    br1b_d = nc.dram_tensor("br1b", [128, 64], BF16, kind="ExternalInput")  # [k, m] x2
    bvb_d = nc.dram_tensor("bvb", [128, D], BF16, kind="ExternalInput")  # bcast rows
    bq_d = nc.dram_tensor("bq", [128, 4], F32, kind="ExternalInput")  # [p, hc]
    bk_d = nc.dram_tensor("bk", [128, 4], F32, kind="ExternalInput")
    out_d = nc.dram_tensor("out", [NPAIR, D], F32, kind="ExternalOutput")

    br2 = 0.0  # placeholder; actual value folded on host via dram? -> no: passed below

    with tile.TileContext(nc) as tc:
        import contextlib

        ctx = contextlib.ExitStack()
        with ctx:
            const_pool = ctx.enter_context(tc.tile_pool(name="const", bufs=1))
            ld_pool = ctx.enter_context(tc.tile_pool(name="loads", bufs=3))
            xt_pool = ctx.enter_context(tc.tile_pool(name="xt", bufs=2))
            proj_pool = ctx.enter_context(tc.tile_pool(name="projs", bufs=2))
            mid_pool = ctx.enter_context(tc.tile_pool(name="mid", bufs=2))
            out_pool = ctx.enter_context(tc.tile_pool(name="outs", bufs=2))
            psA = ctx.enter_context(tc.tile_pool(name="psA", bufs=3, space="PSUM"))
            psS = ctx.enter_context(tc.tile_pool(name="psS", bufs=2, space="PSUM"))
            psC = ctx.enter_context(tc.tile_pool(name="psC", bufs=2, space="PSUM"))
            psB = ctx.enter_context(tc.tile_pool(name="psB", bufs=1, space="PSUM"))

            # --- resident constants ---
            wqT = const_pool.tile([128, 4 * D], BF16, tag="wqT")
            wkT = const_pool.tile([128, 4 * D], BF16, tag="wkT")
            wvT = const_pool.tile([128, 4 * D], BF16, tag="wvT")
            wr1T = const_pool.tile([128, 4 * 64], BF16, tag="wr1T")
            wr2T = const_pool.tile([64, 1], BF16, tag="wr2T")
            br1b = const_pool.tile([128, 64], BF16, tag="br1b")
            bvb = const_pool.tile([128, D], BF16, tag="bvb")
            bqs = const_pool.tile([128, 4], F32, tag="bqs")
            bks = const_pool.tile([128, 4], F32, tag="bks")
            def load_consts():
                nc.sync.dma_start(
                    wqT[:].rearrange("p (dc h) -> p dc h", dc=4),
                    wqT_d[:, :].rearrange("(dc p) h -> p dc h", p=128),
                )
                nc.sync.dma_start(
                    wkT[:].rearrange("p (dc h) -> p dc h", dc=4),
                    wkT_d[:, :].rearrange("(dc p) h -> p dc h", p=128),
                )
                nc.sync.dma_start(
                    wvT[:].rearrange("p (dc h) -> p dc h", dc=4),
                    wvT_d[:, :].rearrange("(dc p) h -> p dc h", p=128),
                )
                nc.sync.dma_start(
                    wr1T[:].rearrange("p (hc m) -> p hc m", hc=4),
                    wr1T_d[:, :].rearrange("(hc p) m -> p hc m", p=128),
                )
                nc.sync.dma_start(wr2T[:], wr2T_d[:, :])
                nc.sync.dma_start(br1b[:], br1b_d[:, :])
                nc.sync.dma_start(bvb[:], bvb_d[:, :])
                nc.sync.dma_start(bqs[:], bq_d[:, :])
                nc.sync.dma_start(bks[:], bk_d[:, :])

            def body(_iv=None):
                load_consts()
                for sb in range(n_sb):
                    emit_superblock(sb)

            def emit_superblock(sb):
                # ---------- Phase A: load + cast + transpose ----------
                xts = {}
                for name, src in (("q", xq), ("k", xk), ("v", xv)):
                    src_ap = src[bass.ts(sb, ROWS_SB), :].rearrange(
                        "(r p) d -> p r d", p=128
                    )
                    if cast_dma:
                        xbf = ld_pool.tile([128, 4 * D], BF16, tag=f"xbf{name}")
                        nc.gpsimd.dma_start(
                            xbf[:].rearrange("p (r d) -> p r d", r=4), src_ap
                        )
                    else:
                        xf32 = ld_pool.tile([128, 4 * D], F32, tag=f"xf{name}")
                        nc.sync.dma_start(
                            xf32[:].rearrange("p (r d) -> p r d", r=4), src_ap
                        )
                        xbf = ld_pool.tile([128, 4 * D], BF16, tag=f"xbf{name}")
                        nc.vector.tensor_copy(xbf[:], xf32[:])
                    xt = xt_pool.tile([128, 4 * D], BF16, tag=f"xt{name}")
                    for r in range(4):
                        for dc in range(4):
                            nc.sync.dma_start(
                                xt[:, dc * 512 + r * 128 : dc * 512 + (r + 1) * 128],
                                xbf[:, r * 512 + dc * 128 : r * 512 + (dc + 1) * 128],
                                transpose=True,
                            )
                    xts[name] = xt

                # ---------- Phase B: projections ----------
                qTs = proj_pool.tile([128, 4 * 512], BF16, tag="qTs")
                kTs = proj_pool.tile([128, 4 * 512], BF16, tag="kTs")
                vS = proj_pool.tile([128, 4 * 512], BF16, tag="vS")
                for dst, w_t, x_t, bias_t in (
                    (qTs, wqT, xts["q"], bqs),
                    (kTs, wkT, xts["k"], bks),
                ):
                    for hc in range(4):
                        ps = psA.tile([128, 512], F32, tag="proj")
                        for dc in range(4):
                            nc.tensor.matmul(
                                ps[:],
                                lhsT=w_t[:, dc * 512 + hc * 128 : dc * 512 + (hc + 1) * 128],
                                rhs=x_t[:, dc * 512 : (dc + 1) * 512],
                                start=(dc == 0),
                                stop=(dc == 3),
                            )
                        nc.scalar.activation(
                            dst[:, hc * 512 : (hc + 1) * 512],
                            ps[:],
                            AT.Identity,
                            bias=bias_t[:, hc : hc + 1],
                        )
                # v natural: [rows(2pair)=128, h=512] per r-block
                for r in range(4):
                    ps = psA.tile([128, 512], F32, tag="proj")
                    for dc in range(4):
                        nc.tensor.matmul(
                            ps[:],
                            lhsT=xts["v"][:, dc * 512 + r * 128 : dc * 512 + (r + 1) * 128],
                            rhs=wvT[:, dc * 512 : (dc + 1) * 512],
                            start=(dc == 0),
                            stop=(dc == 3),
                        )
                    # add bv (broadcast tile) while copying PSUM->SBUF
                    nc.vector.tensor_tensor(
                        vS[:, r * 512 : (r + 1) * 512],
                        ps[:],
                        bvb[:],
                        op=ALU.add,
                    )

                # ---------- Phase C: per-pair attention + reduce MLP ----------
                bat = psB.tile([128, 512], F32, tag="batch")
                # regions: wt=[0:64, 0:8], g=[0:64, 8:16], outT=[:, 16:48]
                Zb = mid_pool.tile([64, 8], F32, tag="Zb")
                e2s = []
                for pp in range(4):  # 2-pair groups
                    e2 = mid_pool.tile([64, 128], BF16, tag=f"e2_{pp}")
                    e2s.append(e2)
                ets = []
                for pr in range(SUPER):
                    pp, half = pr // 2, pr % 2
                    smid = psS.tile([64, 128], F32, tag="smid")
                    # S = q'^T(pair) . k^T(pair), contraction over h
                    for hc in range(4):
                        nc.tensor.matmul(
                            smid[:, 0:64],
                            lhsT=qTs[:, hc * 512 + pr * 64 : hc * 512 + (pr + 1) * 64],
                            rhs=kTs[:, hc * 512 + pr * 64 : hc * 512 + (pr + 1) * 64],
                            start=(hc == 0),
                            stop=(hc == 3),
                        )
                    # E = exp(S) (bf16), Z = rowsum(E) (f32)
                    e2 = e2s[pp]
                    nc.scalar.activation(
                        e2[:, half * 64 : (half + 1) * 64],
                        smid[:, 0:64],
                        AT.Exp,
                        accum_out=Zb[:, pr : pr + 1],
                    )
                    if half == 1:
                        et = mid_pool.tile([128, 64], BF16, tag=f"et_{pp}")
                        nc.sync.dma_start(et[:], e2[:], transpose=True)
                        ets.append(et)

                for pr in range(SUPER):
                    pp, half = pr // 2, pr % 2
                    et = ets[pp]
                    # C~^T chunks: [h-chunk 128, q 64]
                    ctp = psC.tile([128, 256], F32, tag="ct")
                    for hc in range(4):
                        nc.tensor.matmul(
                            ctp[:, hc * 64 : (hc + 1) * 64],
                            lhsT=vS[
                                half * 64 : (half + 1) * 64,
                                pp * 512 + hc * 128 : pp * 512 + (hc + 1) * 128,
                            ],
                            rhs=et[half * 64 : (half + 1) * 64, :],
                            start=True,
                            stop=True,
                        )
                    cts = mid_pool.tile([128, 256], BF16, tag="cts")
                    nc.vector.tensor_copy(cts[:], ctp[:])
                    # hid~ = Wr1 @ C~^T + br1 (x) Z  (rank-1 via br1b matmul)
                    smid2 = psS.tile([64, 128], F32, tag="smid")
                    hid = smid2[:, 64:128]
                    for hc in range(4):
                        nc.tensor.matmul(
                            hid,
                            lhsT=wr1T[:, hc * 64 : (hc + 1) * 64],
                            rhs=cts[:, hc * 64 : (hc + 1) * 64],
                            start=(hc == 0),
                            stop=False,
                        )
                    nc.tensor.matmul(
                        hid,
                        lhsT=br1b[half * 64 : (half + 1) * 64, :],
                        rhs=et[half * 64 : (half + 1) * 64, :],
                        start=False,
                        stop=True,
                    )
                    # leaky relu on ScalarE (PSUM -> SBUF, bf16 cast)
                    ys = mid_pool.tile([64, 64], BF16, tag="ys")
                    nc.scalar.activation(ys[:], hid, AT.Lrelu, alpha=LEAK)
                    # w~^T[q] column into batch psum
                    nc.tensor.matmul(
                        bat[0:64, pr : pr + 1],
                        lhsT=ys[:],
                        rhs=wr2T[:],
                        start=True,
                        stop=True,
                    )

                # ---------- batched per-pair scalars ----------
                rT = mid_pool.tile([64, 8], F32, tag="rT")
                nc.vector.reciprocal(rT[:], Zb[:])
                t1 = mid_pool.tile([64, 8], F32, tag="t1")
                nc.vector.scalar_tensor_tensor(
                    t1[:], bat[0:64, 0:8], 1.0, rT[:], op0=ALU.mult, op1=ALU.mult
                )
                wh = mid_pool.tile([64, 8], BF16, tag="wh")
                nc.vector.scalar_tensor_tensor(
                    wh[:], t1[:], float(BR2_VAL[0]), rT[:], op0=ALU.add, op1=ALU.mult
                )

                # g = E^T @ w^  per pair
                for pr in range(SUPER):
                    pp, half = pr // 2, pr % 2
                    nc.tensor.matmul(
                        bat[0:64, 8 + pr : 9 + pr],
                        lhsT=e2s[pp][:, half * 64 : (half + 1) * 64],
                        rhs=wh[:, pr : pr + 1],
                        start=True,
                        stop=True,
                    )
                gS = mid_pool.tile([128, 8], BF16, tag="gS")
                nc.vector.tensor_copy(gS[0:64, :], bat[0:64, 8:16])
                nc.vector.tensor_copy(gS[64:128, :], bat[0:64, 8:16])

                # out^T chunks
                for pr in range(SUPER):
                    pp, half = pr // 2, pr % 2
                    for hc in range(4):
                        nc.tensor.matmul(
                            bat[:, 16 + pr * 4 + hc : 17 + pr * 4 + hc],
                            lhsT=vS[
                                half * 64 : (half + 1) * 64,
                                pp * 512 + hc * 128 : pp * 512 + (hc + 1) * 128,
                            ],
                            rhs=gS[half * 64 : (half + 1) * 64, pr : pr + 1],
                            start=True,
                            stop=True,
                        )
                outTs = out_pool.tile([128, 32], F32, tag="outTs")
                nc.scalar.activation(outTs[:], bat[:, 16:48], AT.Copy)
                outN = out_pool.tile([32, 128], F32, tag="outN")
                for b in range(4):
                    nc.vector.transpose(
                        outN[0:32, b * 32 : (b + 1) * 32],
                        outTs[b * 32 : (b + 1) * 32, 0:32],
                    )
                nc.sync.dma_start(
                    out_d[bass.ts(sb, SUPER), :].rearrange(
                        "pr (hc c) -> (pr hc) c", hc=4
                    ),
                    outN[:],
                )

            if repeat == 1:
                body()
            else:
                with tc.For_i(0, repeat, 1) as _iv:
                    body(_iv)

    nc.compile()
    return nc


# br2 handled as a python constant captured at build time
BR2_VAL = [0.0]


def prep_in_maps(query, key, value, Wq, bq, Wk, bk, Wv, bv, Wr1, br1, Wr2, br2):
    """Host-side prep: shard + weight transforms. Returns in_maps list of 8 dicts."""
    s = 1.0 / np.sqrt(np.float32(D))
    wqT = (Wq * s).T.astype(BF16_NP).copy()  # [d, h]
    wkT = Wk.T.astype(BF16_NP).copy()
    wvT = Wv.T.astype(BF16_NP).copy()
    wr1T = Wr1.T.astype(BF16_NP).copy()  # [h, m]
    wr2T = Wr2.T.astype(BF16_NP).copy()  # [m, 1]
    br1b = np.tile(br1[None, :].astype(np.float32), (128, 1)).astype(BF16_NP)
    bvb = np.tile(bv[None, :].astype(np.float32), (128, 1)).astype(BF16_NP)
    bqv = (bq * s).astype(np.float32).reshape(4, 128).T.copy()  # [p, hc]
    bkv = bk.astype(np.float32).reshape(4, 128).T.copy()
    BR2_VAL[0] = float(br2[0])

    in_maps = []
    for c in range(NCORES):
        sl = slice(c * BS_SH, (c + 1) * BS_SH)
        in_maps.append(
            {
                "xq": np.ascontiguousarray(query[sl]).reshape(NPAIR * NSHOT, D),
                "xk": np.ascontiguousarray(key[sl]).reshape(NPAIR * NSHOT, D),
                "xv": np.ascontiguousarray(value[sl]).reshape(NPAIR * NSHOT, D),
                "wqT": wqT,
                "wkT": wkT,
                "wvT": wvT,
                "wr1T": wr1T,
                "wr2T": wr2T,
                "br1b": br1b,
                "bvb": bvb,
                "bq": bqv,
                "bk": bkv,
            }
        )
    return in_maps


_nc_cache = {}


def kernel(**inputs):
    in_maps = prep_in_maps(**{k: np.asarray(v) for k, v in inputs.items()})
    key = ("k", 1, BR2_VAL[0])
    if key not in _nc_cache:
        _nc_cache[key] = build_nc(repeat=1)
    nc = _nc_cache[key]
    res = run_bass_kernel_spmd(nc, in_maps, core_ids=list(range(NCORES)))
    outs = [res.results[c]["out"].reshape(BS_SH, NWAY, D) for c in range(NCORES)]
    return np.concatenate(outs, axis=0).astype(np.float32)


if __name__ == "__main__":
    rng = np.random.default_rng(0)
    sh = (BS, NWAY, NSHOT, D)
    ins = {
        "query": rng.normal(size=sh).astype(np.float32),
        "key": rng.normal(size=sh).astype(np.float32),
        "value": rng.normal(size=sh).astype(np.float32),
        "Wq": (rng.normal(size=(D, D)) * 0.02).astype(np.float32),
        "bq": (rng.normal(size=(D,)) * 0.02).astype(np.float32),
        "Wk": (rng.normal(size=(D, D)) * 0.02).astype(np.float32),
        "bk": (rng.normal(size=(D,)) * 0.02).astype(np.float32),
        "Wv": (rng.normal(size=(D, D)) * 0.02).astype(np.float32),
        "bv": (rng.normal(size=(D,)) * 0.02).astype(np.float32),
        "Wr1": (rng.normal(size=(64, D)) * 0.02).astype(np.float32),
        "br1": (rng.normal(size=(64,)) * 0.02).astype(np.float32),
        "Wr2": (rng.normal(size=(1, 64)) * 0.02).astype(np.float32),
        "br2": (rng.normal(size=(1,)) * 0.02).astype(np.float32),
    }
    out = kernel(**ins)
    print("out", out.shape, out.dtype, float(np.abs(out).mean()))


# revision 20
# speedup vs baseline: 1.0422x; 1.0422x over previous
"""Trainium2 Bass kernel for nn_Attention_55499567399068.

Episode-attention block: per (batch, nway) pair of [64, 512] blocks:
  q/k/v linear projections -> scaled dot-product attention over nshot ->
  reduce_att MLP producing per-row weights -> weighted sum of context rows.

Sharding: pure data parallel over batch across 8 NeuronCores (32 episodes each).
Per core: 256 independent (b, n) pairs, processed in 32 superblocks of 8 pairs.

Math restructuring (exact, up to fp precision):
  - 1/sqrt(d) folded into Wq, bq on host.
  - Softmax without max-subtraction (scores are O(1)): E = exp(S) on ScalarE
    with the row-sum Z as the same instruction's accum_out; A = E * (1/Z) is a
    cheap per-partition scale in E-natural layout.
       hid = Wr1 @ C^T; leaky+br1-bias fused into one ScalarE op
       w = hid^T @ Wr2 + br2;  g = A^T @ w;  out^T[h] = sum_k v[k,h] * g[k]
  - bq, bk added via per-partition activation bias on the PSUM->SBUF copy
    (q^T/k^T layouts have h on partitions); bv added via a broadcast-tile
    tensor_tensor add on the v copy (v natural layout has h on free).

Layout strategy: inputs are loaded naturally ([row, d]), cast to bf16, and
transposed to [d, row] with the DMA xbar (2-byte transpose engine) so the
d-contraction projections run at full PE rate.
"""

import sys

sys.path.insert(0, "/opt/trn_rl_repo")

import ml_dtypes
import numpy as np

import concourse.bass as bass
import concourse.tile as tile
from concourse import bacc, mybir
from concourse.bass_utils import run_bass_kernel_spmd

F32 = mybir.dt.float32
BF16 = mybir.dt.bfloat16
BF16_NP = ml_dtypes.bfloat16

BS, NWAY, NSHOT, D = 256, 8, 64, 512
NCORES = 8
BS_SH = BS // NCORES  # 32 episodes per core
NPAIR = BS_SH * NWAY  # 256 pairs per core
SUPER = 8  # pairs per superblock
NSB = NPAIR // SUPER  # 32 superblocks
ROWS_SB = SUPER * NSHOT  # 512 rows per superblock
LEAK = 0.01
AT = mybir.ActivationFunctionType
ALU = mybir.AluOpType

BR2_VAL = [0.0]  # captured at build time as an immediate
SUB = {"act3d": True, "br1grp": True, "widelrelu": True, "zrowmm": True, "rank1": True}


def build_nc(repeat=1, cast_dma=True, n_sb=NSB, lrelu=True, widehid=False, xbar3d=False):
    nc = bacc.Bacc("TRN2", target_bir_lowering=False)

    xq = nc.dram_tensor("xq", [NPAIR * NSHOT, D], F32, kind="ExternalInput")
    xk = nc.dram_tensor("xk", [NPAIR * NSHOT, D], F32, kind="ExternalInput")
    xv = nc.dram_tensor("xv", [NPAIR * NSHOT, D], F32, kind="ExternalInput")
    wqT_d = nc.dram_tensor("wqT", [D, D], BF16, kind="ExternalInput")  # [d, h]
    wkT_d = nc.dram_tensor("wkT", [D, D], BF16, kind="ExternalInput")
    wvT_d = nc.dram_tensor("wvT", [D, D], BF16, kind="ExternalInput")
    wr1T_d = nc.dram_tensor("wr1T", [D, 64], BF16, kind="ExternalInput")  # [h, m]
    wr2T_d = nc.dram_tensor("wr2T", [64, 1], BF16, kind="ExternalInput")  # [m, 1]
    br1b_d = nc.dram_tensor("br1b", [128, 64], BF16, kind="ExternalInput")
    bvb_d = nc.dram_tensor("bvb", [128, D], BF16, kind="ExternalInput")
    bq_d = nc.dram_tensor("bq", [128, 4], F32, kind="ExternalInput")
    bk_d = nc.dram_tensor("bk", [128, 4], F32, kind="ExternalInput")
    br1c_d = nc.dram_tensor("br1c", [64, 1], F32, kind="ExternalInput")
    out_d = nc.dram_tensor("out", [NPAIR, D], F32, kind="ExternalOutput")

    with tile.TileContext(nc) as tc:
        import contextlib

        ctx = contextlib.ExitStack()
        with ctx:
            const_pool = ctx.enter_context(tc.tile_pool(name="const", bufs=1))
            ld_pool = ctx.enter_context(tc.tile_pool(name="loads", bufs=3))
            xt_pool = ctx.enter_context(tc.tile_pool(name="xt", bufs=2))
            proj_pool = ctx.enter_context(tc.tile_pool(name="projs", bufs=2))
            mid_pool = ctx.enter_context(tc.tile_pool(name="mid", bufs=2))
            out_pool = ctx.enter_context(tc.tile_pool(name="outs", bufs=2))
            psA = ctx.enter_context(tc.tile_pool(name="psA", bufs=2, space="PSUM"))
            psS = ctx.enter_context(tc.tile_pool(name="psS", bufs=3, space="PSUM"))
            psC = ctx.enter_context(tc.tile_pool(name="psC", bufs=2, space="PSUM"))
            psB = ctx.enter_context(tc.tile_pool(name="psB", bufs=1, space="PSUM"))

            wqT = const_pool.tile([128, 4 * D], BF16, tag="wqT")
            wkT = const_pool.tile([128, 4 * D], BF16, tag="wkT")
            wvT = const_pool.tile([128, 4 * D], BF16, tag="wvT")
            wr1T = const_pool.tile([128, 4 * 64], BF16, tag="wr1T")
            wr2T = const_pool.tile([64, 1], BF16, tag="wr2T")
            br1b = const_pool.tile([128, 64], BF16, tag="br1b")
            bvb = const_pool.tile([128, D], BF16, tag="bvb")
            bqs = const_pool.tile([128, 4], F32, tag="bqs")
            bks = const_pool.tile([128, 4], F32, tag="bks")
            br1c = const_pool.tile([64, 1], F32, tag="br1c")

            def load_consts():
                nc.sync.dma_start(
                    wqT[:].rearrange("p (dc h) -> p dc h", dc=4),
                    wqT_d[:, :].rearrange("(dc p) h -> p dc h", p=128),
                )
                nc.sync.dma_start(
                    wkT[:].rearrange("p (dc h) -> p dc h", dc=4),
                    wkT_d[:, :].rearrange("(dc p) h -> p dc h", p=128),
                )
                nc.sync.dma_start(
                    wvT[:].rearrange("p (dc h) -> p dc h", dc=4),
                    wvT_d[:, :].rearrange("(dc p) h -> p dc h", p=128),
                )
                nc.sync.dma_start(
                    wr1T[:].rearrange("p (hc m) -> p hc m", hc=4),
                    wr1T_d[:, :].rearrange("(hc p) m -> p hc m", p=128),
                )
                nc.sync.dma_start(wr2T[:], wr2T_d[:, :])
                nc.sync.dma_start(br1b[:], br1b_d[:, :])
                nc.sync.dma_start(bvb[:], bvb_d[:, :])
                nc.sync.dma_start(bqs[:], bq_d[:, :])
                nc.sync.dma_start(bks[:], bk_d[:, :])
                nc.sync.dma_start(br1c[:], br1c_d[:, :])

            def emit_superblock(sb):
                # ---------- Phase A: load + cast + transpose ----------
                xts = {}
                for name, src in (("q", xq), ("k", xk), ("v", xv)):
                    src_ap = src[bass.ts(sb, ROWS_SB), :].rearrange(
                        "(r p) d -> p r d", p=128
                    )
                    if cast_dma:
                        xbf = ld_pool.tile([128, 4 * D], BF16, tag=f"xbf{name}")
                        nc.gpsimd.dma_start(
                            xbf[:].rearrange("p (r d) -> p r d", r=4), src_ap
                        )
                    else:
                        xf32 = ld_pool.tile([128, 4 * D], F32, tag=f"xf{name}")
                        nc.sync.dma_start(
                            xf32[:].rearrange("p (r d) -> p r d", r=4), src_ap
                        )
                        xbf = ld_pool.tile([128, 4 * D], BF16, tag=f"xbf{name}")
                        nc.vector.tensor_copy(xbf[:], xf32[:])
                    xt = xt_pool.tile([128, 4 * D], BF16, tag=f"xt{name}")
                    if xbar3d:
                        xt4 = xt[:].rearrange("p (dc rb i) -> p dc rb i", dc=4, rb=4)
                        for r in range(4):
                            # one xbar DMA transposes a [128, 512] row-block:
                            # out[p, dc, i] = in[i, dc*128+p]
                            nc.sync.dma_start(
                                xt4[:, :, r, :],
                                xbf[:, r * 512 : (r + 1) * 512],
                                transpose=True,
                            )
                    else:
                        for r in range(4):
                            for dc in range(4):
                                nc.sync.dma_start(
                                    xt[
                                        :,
                                        dc * 512 + r * 128 : dc * 512 + (r + 1) * 128,
                                    ],
                                    xbf[
                                        :,
                                        r * 512 + dc * 128 : r * 512 + (dc + 1) * 128,
                                    ],
                                    transpose=True,
                                )
                    xts[name] = xt

                # ---------- Phase B: projections ----------
                qTs = proj_pool.tile([128, 4 * 512], BF16, tag="qTs")
                kTs = proj_pool.tile([128, 4 * 512], BF16, tag="kTs")
                vS = proj_pool.tile([128, 4 * 512], BF16, tag="vS")
                for dst, w_t, x_t, bias_t in (
                    (qTs, wqT, xts["q"], bqs),
                    (kTs, wkT, xts["k"], bks),
                ):
                    for hc in range(4):
                        ps = psA.tile([128, 512], F32, tag="proj")
                        for dc in range(4):
                            nc.tensor.matmul(
                                ps[:],
                                lhsT=w_t[
                                    :, dc * 512 + hc * 128 : dc * 512 + (hc + 1) * 128
                                ],
                                rhs=x_t[:, dc * 512 : (dc + 1) * 512],
                                start=(dc == 0),
                                stop=(dc == 3),
                            )
                        nc.scalar.activation(
                            dst[:, hc * 512 : (hc + 1) * 512],
                            ps[:],
                            AT.Identity,
                            bias=bias_t[:, hc : hc + 1],
                        )
                for r in range(4):
                    ps = psA.tile([128, 512], F32, tag="proj")
                    for dc in range(4):
                        nc.tensor.matmul(
                            ps[:],
                            lhsT=xts["v"][
                                :, dc * 512 + r * 128 : dc * 512 + (r + 1) * 128
                            ],
                            rhs=wvT[:, dc * 512 : (dc + 1) * 512],
                            start=(dc == 0),
                            stop=(dc == 3),
                        )
                    nc.vector.tensor_tensor(
                        vS[:, r * 512 : (r + 1) * 512], ps[:], bvb[:], op=ALU.add
                    )

                # ---------- Phase C ----------
                bat = psB.tile([128, 512], F32, tag="batch")
                Zb = mid_pool.tile([64, 8], F32, tag="Zb")

                # C1: scores, 8 pairs as regions of one bank
                s_all = psS.tile([64, 512], F32, tag="sh")
                for pr in range(SUPER):
                    for hc in range(4):
                        nc.tensor.matmul(
                            s_all[:, pr * 64 : (pr + 1) * 64],
                            lhsT=qTs[:, hc * 512 + pr * 64 : hc * 512 + (pr + 1) * 64],
                            rhs=kTs[:, hc * 512 + pr * 64 : hc * 512 + (pr + 1) * 64],
                            start=(hc == 0),
                            stop=(hc == 3),
                        )
                # C2: E = exp(S), Z = rowsum(E)
                e2s = []
                for pp in range(4):
                    e2 = mid_pool.tile([64, 128], BF16, tag=f"e2_{pp}")
                    e2s.append(e2)
                for pr in range(SUPER):
                    pp, half = pr // 2, pr % 2
                    nc.scalar.activation(
                        e2s[pp][:, half * 64 : (half + 1) * 64],
                        s_all[:, pr * 64 : (pr + 1) * 64],
                        AT.Exp,
                        accum_out=Zb[:, pr : pr + 1],
                    )
                # C2b: normalize in place: A = E * (1/Z), per-partition scale
                rT = mid_pool.tile([64, 8], F32, tag="rT")
                nc.vector.reciprocal(rT[:], Zb[:])
                for pr in range(SUPER):
                    pp, half = pr // 2, pr % 2
                    nc.vector.tensor_scalar(
                        e2s[pp][:, half * 64 : (half + 1) * 64],
                        e2s[pp][:, half * 64 : (half + 1) * 64],
                        rT[:, pr : pr + 1],
                        None,
                        op0=ALU.mult,
                    )
                # C3: A^T via xbar
                ets = []
                for pp in range(4):
                    et = mid_pool.tile([128, 64], BF16, tag=f"et_{pp}")
                    nc.sync.dma_start(et[:], e2s[pp][:], transpose=True)
                    ets.append(et)

                ys_all = mid_pool.tile([64, 512], BF16, tag="ys_all")
                lr = AT.Lrelu if lrelu else AT.Relu
                if widehid:
                    # C4: C~^T per pair into combined [p, hc, (pr q)] tile
                    cts_all = mid_pool.tile([128, 4 * 512], BF16, tag="cts_all")
                    cts3 = cts_all[:].rearrange("p (hc prq) -> p hc prq", hc=4)
                    for pr in range(SUPER):
                        pp, half = pr // 2, pr % 2
                        ctp = psC.tile([128, 256], F32, tag="ct")
                        for hc in range(4):
                            nc.tensor.matmul(
                                ctp[:, hc * 64 : (hc + 1) * 64],
                                lhsT=vS[
                                    half * 64 : (half + 1) * 64,
                                    pp * 512 + hc * 128 : pp * 512 + (hc + 1) * 128,
                                ],
                                rhs=ets[pp][half * 64 : (half + 1) * 64, :],
                                start=True,
                                stop=True,
                            )
                        dst = cts3[:, :, pr * 64 : (pr + 1) * 64]
                        src3 = ctp[:].rearrange("p (hc q) -> p hc q", hc=4)
                        if pr % 2 == 0 or not SUB["act3d"]:
                            nc.vector.tensor_copy(dst, src3)
                        else:
                            nc.scalar.activation(dst, src3, AT.Copy)
                    # C5: hid = 4 wide matmuls (A is normalized, C is true context)
                    hid_all = psS.tile([64, 512], F32, tag="sh")
                    for hc in range(4):
                        nc.tensor.matmul(
                            hid_all[:],
                            lhsT=wr1T[:, hc * 64 : (hc + 1) * 64],
                            rhs=cts_all[:, hc * 512 : (hc + 1) * 512],
                            start=(hc == 0),
                            stop=(hc == 3),
                        )
                    # C6: leaky relu (+ br1 per-partition bias) over all pairs
                    nc.scalar.activation(
                        ys_all[:], hid_all[:], lr, bias=br1c[:], alpha=LEAK
                    )
                else:
                    # v1-style per-pair C~/hid/leaky
                    for pr in range(SUPER):
                        pp, half = pr // 2, pr % 2
                        ctp = psC.tile([128, 256], F32, tag="ct")
                        for hc in range(4):
                            nc.tensor.matmul(
                                ctp[:, hc * 64 : (hc + 1) * 64],
                                lhsT=vS[
                                    half * 64 : (half + 1) * 64,
                                    pp * 512 + hc * 128 : pp * 512 + (hc + 1) * 128,
                                ],
                                rhs=ets[pp][half * 64 : (half + 1) * 64, :],
                                start=True,
                                stop=True,
                            )
                        cts = mid_pool.tile([128, 256], BF16, tag="cts")
                        nc.vector.tensor_copy(cts[:], ctp[:])
                        hid_t = psS.tile([64, 512], F32, tag="sh")
                        hid = hid_t[:, 0:64]
                        for hc in range(4):
                            nc.tensor.matmul(
                                hid,
                                lhsT=wr1T[:, hc * 64 : (hc + 1) * 64],
                                rhs=cts[:, hc * 64 : (hc + 1) * 64],
                                start=(hc == 0),
                                stop=(hc == 3),
                            )
                        nc.scalar.activation(
                            ys_all[:, pr * 64 : (pr + 1) * 64], hid, lr,
                            bias=br1c[:], alpha=LEAK,
                        )

                # C7: w~^T columns
                for pr in range(SUPER):
                    nc.tensor.matmul(
                        bat[0:64, pr : pr + 1],
                        lhsT=ys_all[:, pr * 64 : (pr + 1) * 64],
                        rhs=wr2T[:],
                        start=True,
                        stop=True,
                    )
                # C8: w = w~ + br2 (A was normalized, so no Z scaling here)
                wh = mid_pool.tile([64, 8], BF16, tag="wh")
                nc.vector.tensor_scalar(
                    wh[:], bat[0:64, 0:8], float(BR2_VAL[0]), None, op0=ALU.add
                )
                # C9: g = E^T @ w^
                for pr in range(SUPER):
                    pp, half = pr // 2, pr % 2
                    nc.tensor.matmul(
                        bat[0:64, 8 + pr : 9 + pr],
                        lhsT=e2s[pp][:, half * 64 : (half + 1) * 64],
                        rhs=wh[:, pr : pr + 1],
                        start=True,
                        stop=True,
                    )
                gS = mid_pool.tile([128, 8], BF16, tag="gS")
                nc.vector.tensor_copy(gS[0:64, :], bat[0:64, 8:16])
                nc.vector.tensor_copy(gS[64:128, :], bat[0:64, 8:16])
                # C10: out^T chunks
                for pr in range(SUPER):
                    pp, half = pr // 2, pr % 2
                    for hc in range(4):
                        nc.tensor.matmul(
                            bat[:, 16 + pr * 4 + hc : 17 + pr * 4 + hc],
                            lhsT=vS[
                                half * 64 : (half + 1) * 64,
                                pp * 512 + hc * 128 : pp * 512 + (hc + 1) * 128,
                            ],
                            rhs=gS[half * 64 : (half + 1) * 64, pr : pr + 1],
                            start=True,
                            stop=True,
                        )
                outTs = out_pool.tile([128, 32], F32, tag="outTs")
                nc.scalar.activation(outTs[:], bat[:, 16:48], AT.Copy)
                outN = out_pool.tile([32, 128], F32, tag="outN")
                for b in range(4):
                    nc.vector.transpose(
                        outN[0:32, b * 32 : (b + 1) * 32],
                        outTs[b * 32 : (b + 1) * 32, 0:32],
                    )
                nc.sync.dma_start(
                    out_d[bass.ts(sb, SUPER), :].rearrange(
                        "pr (hc c) -> (pr hc) c", hc=4
                    ),
                    outN[:],
                )

            def body(_iv=None):
                load_consts()
                for sb in range(n_sb):
                    emit_superblock(sb)

            if repeat == 1:
                body()
            else:
                with tc.For_i(0, repeat, 1) as _iv:
                    body(_iv)

    nc.compile()
    return nc


def prep_in_maps(query, key, value, Wq, bq, Wk, bk, Wv, bv, Wr1, br1, Wr2, br2):
    """Host-side prep: shard + weight transforms. Returns in_maps list of 8 dicts."""
    s = 1.0 / np.sqrt(np.float32(D))
    wqT = (Wq * s).T.astype(BF16_NP).copy()  # [d, h]
    wkT = Wk.T.astype(BF16_NP).copy()
    wvT = Wv.T.astype(BF16_NP).copy()
    wr1T = Wr1.T.astype(BF16_NP).copy()  # [h, m]
    wr2T = Wr2.T.astype(BF16_NP).copy()  # [m, 1]
    br1b = np.tile(br1[None, :].astype(np.float32), (128, 1)).astype(BF16_NP)
    bvb = np.tile(bv[None, :].astype(np.float32), (128, 1)).astype(BF16_NP)
    bqv = (bq * s).astype(np.float32).reshape(4, 128).T.copy()  # [p, hc]
    bkv = bk.astype(np.float32).reshape(4, 128).T.copy()
    br1c = br1.astype(np.float32).reshape(64, 1).copy()
    BR2_VAL[0] = float(br2[0])

    in_maps = []
    for c in range(NCORES):
        sl = slice(c * BS_SH, (c + 1) * BS_SH)
        in_maps.append(
            {
                "xq": np.ascontiguousarray(query[sl]).reshape(NPAIR * NSHOT, D),
                "xk": np.ascontiguousarray(key[sl]).reshape(NPAIR * NSHOT, D),
                "xv": np.ascontiguousarray(value[sl]).reshape(NPAIR * NSHOT, D),
                "wqT": wqT,
                "wkT": wkT,
                "wvT": wvT,
                "wr1T": wr1T,
                "wr2T": wr2T,
                "br1b": br1b,
                "bvb": bvb,
                "bq": bqv,
                "bk": bkv,
                "br1c": br1c,
            }
        )
    return in_maps


_nc_cache = {}


def kernel(**inputs):
    in_maps = prep_in_maps(**{k: np.asarray(v) for k, v in inputs.items()})
    key = ("k", 1, BR2_VAL[0])
    if key not in _nc_cache:
        _nc_cache[key] = build_nc(repeat=1)
    nc = _nc_cache[key]
    res = run_bass_kernel_spmd(nc, in_maps, core_ids=list(range(NCORES)))
    outs = [res.results[c]["out"].reshape(BS_SH, NWAY, D) for c in range(NCORES)]
    return np.concatenate(outs, axis=0).astype(np.float32)
